# revision 11
# baseline (speedup 1.0000x reference)
"""Trainium2 Bass kernel for single-token GQA decoder attention.

Problem (hardcoded): B=32, T=1, HIDDEN=2048, 16 q-heads / 4 kv-heads,
head_dim=128, cache len 1024.

Sharding: 8 cores = TP-4 over kv heads x DP-2 over batch. Core c handles
kv head (c % 4) and batches [16*(c//4), 16*(c//4)+16). Each core computes a
partial output [16, 2048] through its wo column shard; the host sums the 4
TP partials per batch group and concatenates the 2 batch groups.

The one-hot cache update is folded algebraically (never materialized):
  logits_s = SCALE*(1-oh_s)*(q.K_s) + SCALE*oh_s*(q.k_new) + mask_s
  out      = sum_s p_s*(1-oh_s)*V_s + (sum_s p_s*oh_s)*v_new
"""

import math
from contextlib import ExitStack

import numpy as np

MAX_SEQ = 1024
NUM_HEADS = 16
NUM_KV_HEADS = 4
HEAD_DIM = 128
HIDDEN = 2048
GROUPS = NUM_HEADS // NUM_KV_HEADS  # 4
EPS = 1e-6
THETA = 1000000.0
SCALE = 1.0 / math.sqrt(HEAD_DIM)
B = 32
N_CORES = 8
TP = NUM_KV_HEADS  # 4
DP = N_CORES // TP  # 2
BL = B // DP  # 16 batches per core
BH = BL * GROUPS  # 64 (batch*head rows per core)
NCHUNK = MAX_SEQ // 128  # 8 s-chunks
KT = HIDDEN // 128  # 16 k-tiles for projections

_NC = None  # cached Bass program


def _build_nc():
    import concourse.bass as bass
    import concourse.tile as tile
    from concourse import mybir

    f32 = mybir.dt.float32
    AF = mybir.ActivationFunctionType
    ALU = mybir.AluOpType

    nc = bass.Bass()

    xT = nc.declare_dram_parameter("xT", [HIDDEN, BL], f32, isOutput=False)[:]
    wqT = nc.declare_dram_parameter("wqT", [HIDDEN, GROUPS * HEAD_DIM], f32, isOutput=False)[:]
    wkT = nc.declare_dram_parameter("wkT", [HIDDEN, HEAD_DIM], f32, isOutput=False)[:]
    wvT = nc.declare_dram_parameter("wvT", [HIDDEN, HEAD_DIM], f32, isOutput=False)[:]
    woT = nc.declare_dram_parameter("woT", [GROUPS * HEAD_DIM, HIDDEN], f32, isOutput=False)[:]
    kcT = nc.declare_dram_parameter("kcT", [BL, HEAD_DIM, MAX_SEQ], f32, isOutput=False)[:]
    vc = nc.declare_dram_parameter("vc", [BL, MAX_SEQ, HEAD_DIM], f32, isOutput=False)[:]
    # cvec rows: 0=oh, 1=1-oh (aoh), 2=SCALE*oh (bvec), 3=mask
    cvec = nc.declare_dram_parameter("cvec", [4, MAX_SEQ], f32, isOutput=False)[:]
    # rope rows: q: wc1, ws1, wc2, ws2 (norm-w folded); k: same
    rope = nc.declare_dram_parameter("rope", [8, HEAD_DIM // 2], f32, isOutput=False)[:]
    esel = nc.declare_dram_parameter("esel", [BL, BH], f32, isOutput=False)[:]
    ident = nc.declare_dram_parameter("ident", [128, 128], f32, isOutput=False)[:]
    outp = nc.declare_dram_parameter("out", [BL, HIDDEN], f32, isOutput=True)[:]

    HALF = HEAD_DIM // 2

    with ExitStack() as ctx:
        tc = ctx.enter_context(tile.TileContext(nc))
        const = ctx.enter_context(tc.tile_pool(name="const", bufs=1))
        work = ctx.enter_context(tc.tile_pool(name="work", bufs=1))
        kpool = ctx.enter_context(tc.tile_pool(name="kpool", bufs=3))
        vpool = ctx.enter_context(tc.tile_pool(name="vpool", bufs=3))
        pp1 = ctx.enter_context(tc.tile_pool(name="pp1", bufs=1, space="PSUM"))
        pp2 = ctx.enter_context(tc.tile_pool(name="pp2", bufs=2, space="PSUM"))

        # ---- constant / weight loads ----
        x_sb = const.tile([128, KT, BL], f32)
        nc.sync.dma_start(out=x_sb, in_=xT.rearrange("(t p) b -> p t b", p=128))
        wq_sb = const.tile([128, KT, GROUPS * HEAD_DIM], f32)
        nc.sync.dma_start(out=wq_sb, in_=wqT.rearrange("(t p) n -> p t n", p=128))
        wk_sb = const.tile([128, KT, HEAD_DIM], f32)
        nc.sync.dma_start(out=wk_sb, in_=wkT.rearrange("(t p) n -> p t n", p=128))
        wv_sb = const.tile([128, KT, HEAD_DIM], f32)
        nc.sync.dma_start(out=wv_sb, in_=wvT.rearrange("(t p) n -> p t n", p=128))
        wo_sb = const.tile([128, GROUPS, HIDDEN], f32)
        nc.sync.dma_start(out=wo_sb, in_=woT.rearrange("(t p) n -> p t n", p=128))
        ident_sb = const.tile([128, 128], f32)
        nc.sync.dma_start(out=ident_sb, in_=ident)
        esel_sb = const.tile([BL, BH], f32)
        nc.sync.dma_start(out=esel_sb, in_=esel)
        # broadcast [64, 1024] copies of oh and aoh
        oh_bc = const.tile([BH, MAX_SEQ], f32)
        nc.sync.dma_start(
            out=oh_bc,
            in_=bass.AP(tensor=cvec.tensor, offset=cvec.offset, ap=[[0, BH], [1, MAX_SEQ]]),
        )
        aoh_bc = const.tile([BH, MAX_SEQ], f32)
        nc.sync.dma_start(
            out=aoh_bc,
            in_=bass.AP(
                tensor=cvec.tensor, offset=cvec.offset + MAX_SEQ, ap=[[0, BH], [1, MAX_SEQ]]
            ),
        )
        # bvec + mask as single rows
        bm_sb = const.tile([1, 2, MAX_SEQ], f32)
        nc.sync.dma_start(
            out=bm_sb,
            in_=bass.AP(
                tensor=cvec.tensor,
                offset=cvec.offset + 2 * MAX_SEQ,
                ap=[[0, 1], [MAX_SEQ, 2], [1, MAX_SEQ]],
            ),
        )
        # rope vectors broadcast over BL partitions
        rope_bc = const.tile([BL, 8, HALF], f32)
        nc.sync.dma_start(
            out=rope_bc,
            in_=bass.AP(tensor=rope.tensor, offset=rope.offset, ap=[[0, BL], [HALF, 8], [1, HALF]]),
        )
        ones_sb = const.tile([1, BH], f32)
        nc.vector.memset(ones_sb, 1.0)
        eps_sb = const.tile([BL, 1], f32)
        nc.vector.memset(eps_sb, float(EPS))

        # ---- projections: Q [16,512], K/V [16,128] ----
        ps_q = pp2.tile([BL, GROUPS * HEAD_DIM], f32, tag="ppa")
        ps_k = pp2.tile([BL, HEAD_DIM], f32, tag="ppb")
        ps_v = pp2.tile([BL, HEAD_DIM], f32, tag="ppb")
        for t in range(KT):
            st = t == 0
            nc.tensor.matmul(ps_q, x_sb[:, t, :], wq_sb[:, t, :], start=st, stop=t == KT - 1)
            nc.tensor.matmul(ps_k, x_sb[:, t, :], wk_sb[:, t, :], start=st, stop=t == KT - 1)
            nc.tensor.matmul(ps_v, x_sb[:, t, :], wv_sb[:, t, :], start=st, stop=t == KT - 1)

        qc = work.tile([BL, GROUPS, HEAD_DIM], f32)
        nc.vector.tensor_copy(qc.rearrange("b g d -> b (g d)"), ps_q)
        kc_new = work.tile([BL, HEAD_DIM], f32)
        nc.vector.tensor_copy(kc_new, ps_k)
        v_new = work.tile([BL, HEAD_DIM], f32)
        nc.vector.tensor_copy(v_new, ps_v)

        # ---- RMSNorm (q_norm_w/k_norm_w folded into rope vecs) ----
        ssq_q = work.tile([BL, GROUPS], f32)
        q2 = work.tile([BL, GROUPS, HEAD_DIM], f32)
        nc.vector.tensor_mul(q2, qc, qc)
        nc.vector.reduce_sum(ssq_q, q2, axis=mybir.AxisListType.X)
        rms_q = work.tile([BL, GROUPS], f32)
        nc.scalar.activation(rms_q, ssq_q, AF.Sqrt, bias=eps_sb, scale=1.0 / HEAD_DIM)
        rinv_q = work.tile([BL, GROUPS], f32)
        nc.vector.reciprocal(rinv_q, rms_q)

        k2 = work.tile([BL, HEAD_DIM], f32)
        nc.vector.tensor_mul(k2, kc_new, kc_new)
        ssq_k = work.tile([BL, 1], f32)
        nc.vector.reduce_sum(ssq_k, k2, axis=mybir.AxisListType.X)
        rms_k = work.tile([BL, 1], f32)
        nc.scalar.activation(rms_k, ssq_k, AF.Sqrt, bias=eps_sb, scale=1.0 / HEAD_DIM)
        rinv_k = work.tile([BL, 1], f32)
        nc.vector.reciprocal(rinv_k, rms_k)

        qn = work.tile([BL, GROUPS, HEAD_DIM], f32)
        for g in range(GROUPS):
            nc.vector.tensor_scalar_mul(qn[:, g, :], qc[:, g, :], rinv_q[:, g : g + 1])
        kn = work.tile([BL, HEAD_DIM], f32)
        nc.vector.tensor_scalar_mul(kn, kc_new, rinv_k)

        # ---- RoPE (head-batched, rope vecs broadcast over head dim) ----
        def rvec(row, nheads):
            return bass.AP(
                tensor=rope_bc.tensor,
                offset=rope_bc.offset + row * HALF,
                ap=[list(rope_bc.ap[0]), [0, nheads], [1, HALF]],
            )

        qr = work.tile([BL, GROUPS, HEAD_DIM], f32)
        x1 = qn[:, :, 0:HALF]
        x2 = qn[:, :, HALF:HEAD_DIM]
        t1 = work.tile([BL, GROUPS, HALF], f32, tag="rtmp1")
        t2 = work.tile([BL, GROUPS, HALF], f32, tag="rtmp2")
        nc.vector.tensor_mul(t1, x1, rvec(0, GROUPS))
        nc.vector.tensor_mul(t2, x2, rvec(3, GROUPS))
        nc.vector.tensor_sub(qr[:, :, 0:HALF], t1, t2)
        t3 = work.tile([BL, GROUPS, HALF], f32, tag="rtmp1")
        t4 = work.tile([BL, GROUPS, HALF], f32, tag="rtmp2")
        nc.vector.tensor_mul(t3, x2, rvec(2, GROUPS))
        nc.vector.tensor_mul(t4, x1, rvec(1, GROUPS))
        nc.vector.tensor_add(qr[:, :, HALF:HEAD_DIM], t3, t4)

        def rvec2(row):
            return bass.AP(
                tensor=rope_bc.tensor,
                offset=rope_bc.offset + row * HALF,
                ap=[list(rope_bc.ap[0]), [1, HALF]],
            )

        kr = work.tile([BL, HEAD_DIM], f32)
        kx1 = kn[:, 0:HALF]
        kx2 = kn[:, HALF:HEAD_DIM]
        kt1 = work.tile([BL, HALF], f32, tag="ktmp1")
        kt2 = work.tile([BL, HALF], f32, tag="ktmp2")
        nc.vector.tensor_mul(kt1, kx1, rvec2(4))
        nc.vector.tensor_mul(kt2, kx2, rvec2(7))
        nc.vector.tensor_sub(kr[:, 0:HALF], kt1, kt2)
        kt3 = work.tile([BL, HALF], f32, tag="ktmp1")
        kt4 = work.tile([BL, HALF], f32, tag="ktmp2")
        nc.vector.tensor_mul(kt3, kx2, rvec2(6))
        nc.vector.tensor_mul(kt4, kx1, rvec2(5))
        nc.vector.tensor_add(kr[:, HALF:HEAD_DIM], kt3, kt4)

        # ---- build masked qT: mq[128 d, 16 b, 64 bh] block-diag layout ----
        # transpose qr per head -> psum qT [128 d, 4 g, 16 b] (one bank)
        ps_qT = pp2.tile([128, GROUPS, BL], f32, tag="ppa")
        for g in range(GROUPS):
            nc.tensor.transpose(ps_qT[:, g, :], qr[:, g, :], ident_sb[0:BL, 0:BL])
        mq = work.tile([128, BL, BH], f32)
        nc.vector.memset(mq.rearrange("p b c -> p (b c)"), 0.0)
        # col of (b, g) within slice b = 4b + g; src col in ps_qT = 16g + b
        for b in range(BL):
            src = bass.AP(
                tensor=ps_qT.tensor,
                offset=ps_qT.offset + b,
                ap=[list(ps_qT.ap[0]), [BL, GROUPS]],
            )
            nc.vector.tensor_copy(mq[:, b, 4 * b : 4 * b + 4], src)

        # kT: [128 d, 16 b]
        ps_kT = pp2.tile([128, BL], f32, tag="ppb")
        nc.tensor.transpose(ps_kT, kr, ident_sb[0:BL, 0:BL])
        kT_sb = work.tile([128, BL], f32)
        nc.vector.tensor_copy(kT_sb, ps_kT)

        # ---- QK logits + t into PSUM [64, 1024] ----
        ps_l = pp1.tile([BH, MAX_SEQ], f32, tag="ps_l")  # 2 banks
        ps_t = pp1.tile([BH, 1], f32, tag="ps_t")
        for j in range(BL // 2):  # 2-batch cache tiles
            kc_sb = kpool.tile([128, 2, MAX_SEQ], f32, tag="kc")
            nc.sync.dma_start(
                out=kc_sb, in_=kcT[2 * j : 2 * j + 2].rearrange("b p s -> p b s")
            )
            for i in range(2):
                b = 2 * j + i
                lhs = mq[:, b, :]
                nc.tensor.matmul(
                    ps_l[:, 0:512], lhs, kc_sb[:, i, 0:512], start=(b == 0), stop=False
                )
                nc.tensor.matmul(
                    ps_l[:, 512:1024], lhs, kc_sb[:, i, 512:1024], start=(b == 0), stop=False
                )
                nc.tensor.matmul(
                    ps_t, lhs, kT_sb[:, b : b + 1], start=(b == 0), stop=(b == BL - 1)
                )
        # + mask (rank-1 via K=1 matmul with ones row)
        nc.tensor.matmul(ps_l[:, 0:512], ones_sb, bm_sb[:, 1, 0:512], start=False, stop=False)
        nc.tensor.matmul(
            ps_l[:, 512:1024], ones_sb, bm_sb[:, 1, 512:1024], start=False, stop=False
        )
        # + t * bvec (rank-1): transpose t [64,1] -> row [1,64]
        t_col = work.tile([BH, 1], f32)
        nc.vector.tensor_copy(t_col, ps_t)
        ps_tr = pp2.tile([1, BH], f32, tag="ppb")
        nc.tensor.transpose(ps_tr, t_col, ident_sb[0:BH, 0:BH])
        t_row = work.tile([1, BH], f32)
        nc.vector.tensor_copy(t_row, ps_tr)
        nc.tensor.matmul(ps_l[:, 0:512], t_row, bm_sb[:, 0, 0:512], start=False, stop=True)
        nc.tensor.matmul(
            ps_l[:, 512:1024], t_row, bm_sb[:, 0, 512:1024], start=False, stop=True
        )

        # ---- softmax ----
        negmax = work.tile([BH, 1], f32)
        nc.vector.tensor_reduce(
            negmax, ps_l, axis=mybir.AxisListType.X, op=ALU.max, negate=True
        )
        et = work.tile([BH, MAX_SEQ], f32)
        ssum = work.tile([BH, 1], f32)
        nc.scalar.activation(et, ps_l, AF.Exp, bias=negmax, scale=1.0, accum_out=ssum)
        rsum = work.tile([BH, 1], f32)
        nc.vector.reciprocal(rsum, ssum)
        p3 = work.tile([BH, MAX_SEQ], f32)
        sp3 = work.tile([BH, 1], f32)
        nc.vector.scalar_tensor_tensor(
            out=p3,
            in0=et,
            scalar=rsum,
            in1=aoh_bc,
            op0=ALU.mult,
            op1=ALU.mult,
            accum_out=sp3,
        )
        # c = 1 - sum(p3)  (the oh-weighted prob mass)
        c_sb = work.tile([BH, 1], f32)
        nc.vector.tensor_scalar(
            out=c_sb, in0=sp3, scalar1=-1.0, scalar2=1.0, op0=ALU.mult, op1=ALU.add
        )

        # ---- transpose p3 -> pT [128 s, 8 c, 64 bh] ----
        pT = work.tile([128, NCHUNK, BH], f32)
        for cch in range(NCHUNK):
            ps_pt = pp2.tile([128, BH], f32, tag="ppa")
            nc.tensor.transpose(
                ps_pt, p3[:, 128 * cch : 128 * (cch + 1)], ident_sb[0:BH, 0:BH]
            )
            nc.vector.tensor_copy(pT[:, cch, :], ps_pt)

        # ---- AV: psum_o [128 d, 64 bh] ----
        ps_o = pp1.tile([128, BH], f32, tag="ps_o")
        for j in range(BL // 2):
            vc_sb = vpool.tile([128, 2, NCHUNK, HEAD_DIM], f32, tag="vc")
            nc.sync.dma_start(
                out=vc_sb, in_=vc[2 * j : 2 * j + 2].rearrange("b (c p) d -> p b c d", p=128)
            )
            for i in range(2):
                b = 2 * j + i
                for cch in range(NCHUNK):
                    nc.tensor.matmul(
                        ps_o[:, 4 * b : 4 * b + 4],
                        vc_sb[:, i, cch, :],
                        pT[:, cch, 4 * b : 4 * b + 4],
                        start=(b == 0 and cch == 0),
                        stop=False,
                    )
        # + c * v_new via selector matmul: first c -> row -> bcast [16, 64]
        ps_cr = pp2.tile([1, BH], f32, tag="ppb")
        nc.tensor.transpose(ps_cr, c_sb, ident_sb[0:BH, 0:BH])
        c_row = work.tile([1, BH], f32)
        nc.vector.tensor_copy(c_row, ps_cr)
        ps_cb = pp2.tile([BL, BH], f32, tag="ppb")
        nc.tensor.matmul(ps_cb, ones_sb[:, 0:BL], c_row, start=True, stop=True)
        rhs_ec = work.tile([BL, BH], f32)
        nc.vector.tensor_mul(rhs_ec, esel_sb, ps_cb)
        nc.tensor.matmul(ps_o, v_new, rhs_ec, start=False, stop=True)

        attnT = work.tile([128, BH], f32)
        nc.vector.tensor_copy(attnT, ps_o)

        # ---- output projection: out[16, 2048] = attn @ woT ----
        out_sb = work.tile([BL, HIDDEN], f32)
        attnT_g = attnT.rearrange("p (b g) -> p g b", g=GROUPS)
        for ncb in range(4):
            ps_out = pp2.tile([BL, 512], f32, tag="ppa")
            for g in range(GROUPS):
                nc.tensor.matmul(
                    ps_out,
                    attnT_g[:, g, :],
                    wo_sb[:, g, 512 * ncb : 512 * (ncb + 1)],
                    start=(g == 0),
                    stop=(g == GROUPS - 1),
                )
            nc.scalar.copy(out_sb[:, 512 * ncb : 512 * (ncb + 1)], ps_out)
        nc.sync.dma_start(out=outp, in_=out_sb)

    return nc


def _legalize_waits(nc, max_waits=1):
    """walrus in this toolchain accepts at most ONE sync wait per hardware
    instruction; hoist extras onto standalone sequencer sem-waits."""
    from concourse import mybir

    n_fix = 0
    for f in nc.m.functions:
        for blk in f.blocks:
            insts = blk.instructions
            i = 0
            while i < len(insts):
                inst = insts[i]
                si = inst.sync_info
                waits = list(si.on_wait) if si is not None else []
                if len(waits) > max_waits:
                    keep = waits[-max_waits:]
                    extra = waits[:-max_waits]
                    for k, w in enumerate(extra):
                        ev = mybir.InstEventSemaphore(
                            name=f"{inst.name}-hw{k}",
                            engine=inst.engine,
                            sync_info=mybir.SyncInfo(on_wait=[w], on_update=[]),
                            ins=[],
                            outs=[],
                        )
                        insts.insert(i, ev)
                        i += 1
                    inst.sync_info = mybir.SyncInfo(
                        on_wait=keep, on_update=list(si.on_update)
                    )
                    n_fix += 1
                i += 1
    return n_fix


def _get_nc():
    global _NC
    if _NC is None:
        _NC = _build_nc()
        _legalize_waits(_NC)
    return _NC


def _host_prep(x, position, mask, k_cache, v_cache, onehot, wq, wk, wv, wo, q_norm_w, k_norm_w):
    """Build the 8 per-core input maps (all numpy, f32)."""
    x = np.asarray(x, np.float32).reshape(B, HIDDEN)
    pos = np.float32(np.asarray(position).reshape(-1)[0])
    m = np.asarray(mask, np.float32).reshape(MAX_SEQ)
    oh = np.asarray(onehot, np.float32).reshape(MAX_SEQ)
    k_cache = np.asarray(k_cache, np.float32)
    v_cache = np.asarray(v_cache, np.float32)
    wq = np.asarray(wq, np.float32)
    wk = np.asarray(wk, np.float32)
    wv = np.asarray(wv, np.float32)
    wo = np.asarray(wo, np.float32)
    qw = np.asarray(q_norm_w, np.float32)
    kw = np.asarray(k_norm_w, np.float32)

    half = HEAD_DIM // 2
    inv_freq = (1.0 / (THETA ** (np.arange(half, dtype=np.float32) / np.float32(half)))).astype(
        np.float32
    )
    freqs = (pos * inv_freq).astype(np.float32)
    cos_v = np.cos(freqs).astype(np.float32)
    sin_v = np.sin(freqs).astype(np.float32)
    # folded rope vectors: out1 = x1*(w1*cos) - x2*(w2*sin); out2 = x2*(w2*cos) + x1*(w1*sin)
    rope_rows = []
    for w in (qw, kw):
        w1, w2 = w[:half], w[half:]
        rope_rows += [w1 * cos_v, w1 * sin_v, w2 * cos_v, w2 * sin_v]
    rope_arr = np.ascontiguousarray(np.stack(rope_rows), np.float32)

    aoh = (1.0 - oh).astype(np.float32)
    cvec = np.ascontiguousarray(
        np.stack([oh, aoh, (SCALE * oh).astype(np.float32), m]), np.float32
    )

    esel = np.zeros((BL, BH), np.float32)
    for b in range(BL):
        esel[b, GROUPS * b : GROUPS * b + GROUPS] = 1.0
    ident = np.eye(128, dtype=np.float32)

    # scale K cache columns by SCALE*(1-oh_s) (folds the blend+scale into QK)
    a_s = (SCALE * aoh).astype(np.float32)

    in_maps = []
    wqT_s, wkT_s, wvT_s, woT_s = [], [], [], []
    for h in range(TP):
        wqT_s.append(np.ascontiguousarray(wq[512 * h : 512 * h + 512, :].T))
        wkT_s.append(np.ascontiguousarray(wk[128 * h : 128 * h + 128, :].T))
        wvT_s.append(np.ascontiguousarray(wv[128 * h : 128 * h + 128, :].T))
        woT_s.append(np.ascontiguousarray(wo[:, 512 * h : 512 * h + 512].T))
    for core in range(N_CORES):
        h = core % TP
        g = core // TP
        bs = slice(BL * g, BL * g + BL)
        kcT = np.ascontiguousarray(
            k_cache[bs, h].transpose(0, 2, 1) * a_s[None, None, :]
        ).astype(np.float32)
        vcs = np.ascontiguousarray(v_cache[bs, h])
        in_maps.append(
            {
                "xT": np.ascontiguousarray(x[bs].T),
                "wqT": wqT_s[h],
                "wkT": wkT_s[h],
                "wvT": wvT_s[h],
                "woT": woT_s[h],
                "kcT": kcT,
                "vc": vcs,
                "cvec": cvec,
                "rope": rope_arr,
                "esel": esel,
                "ident": ident,
            }
        )
    return in_maps


def _combine(results):
    """Sum TP partials within each batch group, concat groups."""
    out = np.zeros((B, HIDDEN), np.float32)
    for core in range(N_CORES):
        g = core // TP
        out[BL * g : BL * g + BL] += results[core]["out"]
    return out.reshape(B, 1, HIDDEN)


def run_on_cores(in_maps, trace=False, **kw):
    from concourse.bass_utils import run_bass_kernel_spmd

    nc = _get_nc()
    return run_bass_kernel_spmd(nc, in_maps, core_ids=list(range(N_CORES)), trace=trace, **kw)


def kernel(**inputs):
    in_maps = _host_prep(**inputs)
    res = run_on_cores(in_maps)
    return _combine(res.results)


# revision 23
# speedup vs baseline: 1.3536x; 1.3536x over previous
"""Trainium2 Bass kernel for single-token GQA decoder attention.

Problem (hardcoded): B=32, T=1, HIDDEN=2048, 16 q-heads / 4 kv-heads,
head_dim=128, cache len 1024.

Sharding: 8 cores = TP-4 over kv heads x DP-2 over batch. Core c handles
kv head (c % 4) and batches [16*(c//4), 16*(c//4)+16). Each core computes a
partial output [16, 2048] through its wo column shard; the host sums the 4
TP partials per batch group and concatenates the 2 batch groups.

The one-hot cache update is folded algebraically (never materialized):
  logits_s = SCALE*(1-oh_s)*(q.K_s) + SCALE*oh_s*(q.k_new) + mask_s
  out      = sum_s p_s*(1-oh_s)*V_s + (sum_s p_s*oh_s)*v_new
"""

import math
from contextlib import ExitStack

import numpy as np

MAX_SEQ = 1024
NUM_HEADS = 16
NUM_KV_HEADS = 4
HEAD_DIM = 128
HIDDEN = 2048
GROUPS = NUM_HEADS // NUM_KV_HEADS  # 4
EPS = 1e-6
THETA = 1000000.0
SCALE = 1.0 / math.sqrt(HEAD_DIM)
B = 32
N_CORES = 8
TP = NUM_KV_HEADS  # 4
DP = N_CORES // TP  # 2
BL = B // DP  # 16 batches per core
BH = BL * GROUPS  # 64 (batch*head rows per core)
NCHUNK = MAX_SEQ // 128  # 8 s-chunks
KT = HIDDEN // 128  # 16 k-tiles for projections

_NC = None  # cached Bass program


def _build_nc():
    import concourse.bass as bass
    import concourse.tile as tile
    from concourse import mybir

    f32 = mybir.dt.float32
    AF = mybir.ActivationFunctionType
    ALU = mybir.AluOpType

    nc = bass.Bass()

    xT = nc.declare_dram_parameter("xT", [HIDDEN, BL], f32, isOutput=False)[:]
    wqT = nc.declare_dram_parameter("wqT", [HIDDEN, GROUPS * HEAD_DIM], f32, isOutput=False)[:]
    wkT = nc.declare_dram_parameter("wkT", [HIDDEN, HEAD_DIM], f32, isOutput=False)[:]
    wvT = nc.declare_dram_parameter("wvT", [HIDDEN, HEAD_DIM], f32, isOutput=False)[:]
    woT = nc.declare_dram_parameter("woT", [GROUPS * HEAD_DIM, HIDDEN], f32, isOutput=False)[:]
    kcT = nc.declare_dram_parameter("kcT", [BL, HEAD_DIM, MAX_SEQ], f32, isOutput=False)[:]
    vc = nc.declare_dram_parameter("vc", [BL, MAX_SEQ, HEAD_DIM], f32, isOutput=False)[:]
    # cvec rows: 0=oh, 1=1-oh (aoh), 2=SCALE*oh (bvec), 3=mask
    cvec = nc.declare_dram_parameter("cvec", [4, MAX_SEQ], f32, isOutput=False)[:]
    # rope rows: q: wc1, ws1, wc2, ws2 (norm-w folded); k: same
    rope = nc.declare_dram_parameter("rope", [8, HEAD_DIM // 2], f32, isOutput=False)[:]
    esel = nc.declare_dram_parameter("esel", [BL, BH], f32, isOutput=False)[:]
    ident = nc.declare_dram_parameter("ident", [128, 128], f32, isOutput=False)[:]
    outp = nc.declare_dram_parameter("out", [BL, HIDDEN], f32, isOutput=True)[:]

    HALF = HEAD_DIM // 2

    with ExitStack() as ctx:
        tc = ctx.enter_context(tile.TileContext(nc))
        const = ctx.enter_context(tc.tile_pool(name="const", bufs=1))
        work = ctx.enter_context(tc.tile_pool(name="work", bufs=1))
        kpool = ctx.enter_context(tc.tile_pool(name="kpool", bufs=2))
        vpool = ctx.enter_context(tc.tile_pool(name="vpool", bufs=2))
        pp = ctx.enter_context(tc.tile_pool(name="pp", bufs=1, space="PSUM"))

        # ---- constant / weight loads ----
        f32r = mybir.dt.float32r
        x_sb = const.tile([128, KT, BL], f32r)
        nc.sync.dma_start(out=x_sb, in_=xT.rearrange("(t p) b -> p t b", p=128).bitcast(f32r))
        wq_sb = const.tile([128, KT, GROUPS * HEAD_DIM], f32r)
        nc.sync.dma_start(out=wq_sb, in_=wqT.rearrange("(t p) n -> p t n", p=128).bitcast(f32r))
        wk_sb = const.tile([128, KT, HEAD_DIM], f32r)
        nc.sync.dma_start(out=wk_sb, in_=wkT.rearrange("(t p) n -> p t n", p=128).bitcast(f32r))
        wv_sb = const.tile([128, KT, HEAD_DIM], f32r)
        nc.sync.dma_start(out=wv_sb, in_=wvT.rearrange("(t p) n -> p t n", p=128).bitcast(f32r))
        ident_sb = const.tile([128, 128], f32)
        nc.sync.dma_start(out=ident_sb, in_=ident)
        esel_sb = const.tile([BL, BH], f32)
        nc.sync.dma_start(out=esel_sb, in_=esel)
        # broadcast [64, 1024] copy of aoh = 1-onehot
        aoh_bc = const.tile([BH, MAX_SEQ], f32)
        nc.sync.dma_start(
            out=aoh_bc,
            in_=bass.AP(
                tensor=cvec.tensor, offset=cvec.offset + MAX_SEQ, ap=[[0, BH], [1, MAX_SEQ]]
            ),
        )
        # bvec (partition 0) + mask (partition 1) as a [2, S] tile
        bm2_sb = const.tile([2, MAX_SEQ], f32r)
        nc.sync.dma_start(out=bm2_sb, in_=cvec[2:4, :].bitcast(f32r))
        # rope vectors broadcast over BL partitions
        rope_bc = const.tile([BL, 8, HALF], f32)
        nc.sync.dma_start(
            out=rope_bc,
            in_=bass.AP(tensor=rope.tensor, offset=rope.offset, ap=[[0, BL], [HALF, 8], [1, HALF]]),
        )
        eps_sb = const.tile([BL, 1], f32)
        nc.vector.memset(eps_sb, float(EPS))
        ones_sb = const.tile([1, BH], f32)
        nc.vector.memset(ones_sb, 1.0)

        # ---- projections: Q [16,512], K/V [16,128] ----
        ps_q = pp.tile([BL, GROUPS * HEAD_DIM], f32, tag="L")
        ps_k = pp.tile([BL, HEAD_DIM], f32, tag="T")
        ps_v = pp.tile([BL, HEAD_DIM], f32, tag="U")
        for t in range(KT):
            st = t == 0
            nc.tensor.matmul(ps_q, x_sb[:, t, :], wq_sb[:, t, :], start=st, stop=t == KT - 1)
            nc.tensor.matmul(ps_k, x_sb[:, t, :], wk_sb[:, t, :], start=st, stop=t == KT - 1)
            nc.tensor.matmul(ps_v, x_sb[:, t, :], wv_sb[:, t, :], start=st, stop=t == KT - 1)

        qc = work.tile([BL, GROUPS, HEAD_DIM], f32)
        nc.vector.tensor_copy(qc.rearrange("b g d -> b (g d)"), ps_q)
        kc_new = work.tile([BL, HEAD_DIM], f32)
        nc.vector.tensor_copy(kc_new, ps_k)
        v_new = work.tile([BL, HEAD_DIM], f32)
        nc.vector.tensor_copy(v_new, ps_v)

        # ---- RMSNorm (q_norm_w/k_norm_w folded into rope vecs) ----
        ssq_q = work.tile([BL, GROUPS], f32)
        q2 = work.tile([BL, GROUPS, HEAD_DIM], f32)
        nc.vector.tensor_mul(q2, qc, qc)
        nc.vector.reduce_sum(ssq_q, q2, axis=mybir.AxisListType.X)
        rms_q = work.tile([BL, GROUPS], f32)
        nc.scalar.activation(rms_q, ssq_q, AF.Sqrt, bias=eps_sb, scale=1.0 / HEAD_DIM)
        rinv_q = work.tile([BL, GROUPS], f32)
        nc.vector.reciprocal(rinv_q, rms_q)

        k2 = work.tile([BL, HEAD_DIM], f32)
        nc.vector.tensor_mul(k2, kc_new, kc_new)
        ssq_k = work.tile([BL, 1], f32)
        nc.vector.reduce_sum(ssq_k, k2, axis=mybir.AxisListType.X)
        rms_k = work.tile([BL, 1], f32)
        nc.scalar.activation(rms_k, ssq_k, AF.Sqrt, bias=eps_sb, scale=1.0 / HEAD_DIM)
        rinv_k = work.tile([BL, 1], f32)
        nc.vector.reciprocal(rinv_k, rms_k)

        qn = work.tile([BL, GROUPS, HEAD_DIM], f32)
        for g in range(GROUPS):
            nc.vector.tensor_scalar_mul(qn[:, g, :], qc[:, g, :], rinv_q[:, g : g + 1])
        kn = work.tile([BL, HEAD_DIM], f32)
        nc.vector.tensor_scalar_mul(kn, kc_new, rinv_k)

        # ---- RoPE (head-batched, rope vecs broadcast over head dim) ----
        def rvec(row, nheads):
            return bass.AP(
                tensor=rope_bc.tensor,
                offset=rope_bc.offset + row * HALF,
                ap=[list(rope_bc.ap[0]), [0, nheads], [1, HALF]],
            )

        qr = work.tile([BL, GROUPS, HEAD_DIM], f32)
        x1 = qn[:, :, 0:HALF]
        x2 = qn[:, :, HALF:HEAD_DIM]
        t1 = work.tile([BL, GROUPS, HALF], f32, tag="rtmp1")
        t2 = work.tile([BL, GROUPS, HALF], f32, tag="rtmp2")
        nc.vector.tensor_mul(t1, x1, rvec(0, GROUPS))
        nc.vector.tensor_mul(t2, x2, rvec(3, GROUPS))
        nc.vector.tensor_sub(qr[:, :, 0:HALF], t1, t2)
        t3 = work.tile([BL, GROUPS, HALF], f32, tag="rtmp1")
        t4 = work.tile([BL, GROUPS, HALF], f32, tag="rtmp2")
        nc.vector.tensor_mul(t3, x2, rvec(2, GROUPS))
        nc.vector.tensor_mul(t4, x1, rvec(1, GROUPS))
        nc.vector.tensor_add(qr[:, :, HALF:HEAD_DIM], t3, t4)

        def rvec2(row):
            return bass.AP(
                tensor=rope_bc.tensor,
                offset=rope_bc.offset + row * HALF,
                ap=[list(rope_bc.ap[0]), [1, HALF]],
            )

        kr = work.tile([BL, HEAD_DIM], f32)
        kx1 = kn[:, 0:HALF]
        kx2 = kn[:, HALF:HEAD_DIM]
        kt1 = work.tile([BL, HALF], f32, tag="ktmp1")
        kt2 = work.tile([BL, HALF], f32, tag="ktmp2")
        nc.vector.tensor_mul(kt1, kx1, rvec2(4))
        nc.vector.tensor_mul(kt2, kx2, rvec2(7))
        nc.vector.tensor_sub(kr[:, 0:HALF], kt1, kt2)
        kt3 = work.tile([BL, HALF], f32, tag="ktmp1")
        kt4 = work.tile([BL, HALF], f32, tag="ktmp2")
        nc.vector.tensor_mul(kt3, kx2, rvec2(6))
        nc.vector.tensor_mul(kt4, kx1, rvec2(5))
        nc.vector.tensor_add(kr[:, HALF:HEAD_DIM], kt3, kt4)

        # ---- build masked qT: mq[128 d, 16 b, 64 bh] block-diag layout ----
        # transpose qr per head -> psum qT [128 d, 4 g, 16 b] (one bank)
        ps_qT = pp.tile([128, GROUPS, BL], f32, tag="U")
        for g in range(GROUPS):
            nc.tensor.transpose(ps_qT[:, g, :], qr[:, g, :], ident_sb[0:BL, 0:BL])
        mq = work.tile([128, BL, BH], f32r)
        nc.vector.memset(mq.rearrange("p b c -> p (b c)").bitcast(f32), 0.0)
        # col of (b, g) within slice b = 4b + g; src col in ps_qT = 16g + b
        for b in range(BL):
            src = bass.AP(
                tensor=ps_qT.tensor,
                offset=ps_qT.offset + b,
                ap=[list(ps_qT.ap[0]), [BL, GROUPS]],
            )
            nc.vector.tensor_copy(mq[:, b, 4 * b : 4 * b + 4], src)

        # kT: [128 d, 16 b]
        ps_kT = pp.tile([128, BL], f32, tag="T")
        nc.tensor.transpose(ps_kT, kr, ident_sb[0:BL, 0:BL])
        kT_sb = work.tile([128, BL], f32r)
        nc.vector.tensor_copy(kT_sb, ps_kT)

        # ---- QK logits + t into PSUM [64, 1024] ----
        ps_l = pp.tile([BH, MAX_SEQ], f32, tag="L")  # 2 banks
        ps_t = pp.tile([BH, 1], f32, tag="T")
        for j in range(BL // 4):  # 4-batch (2 MB) cache tiles
            kc_sb = kpool.tile([128, 4, MAX_SEQ], f32r, tag="kc")
            nc.sync.dma_start(
                out=kc_sb, in_=kcT[4 * j : 4 * j + 4].rearrange("b p s -> p b s").bitcast(f32r)
            )
            for i in range(4):
                b = 4 * j + i
                lhs = mq[:, b, :]
                nc.tensor.matmul(
                    ps_l[:, 0:512], lhs, kc_sb[:, i, 0:512], start=(b == 0), stop=False
                )
                nc.tensor.matmul(
                    ps_l[:, 512:1024], lhs, kc_sb[:, i, 512:1024], start=(b == 0), stop=False
                )
                nc.tensor.matmul(
                    ps_t,
                    lhs.bitcast(f32),
                    kT_sb[:, b : b + 1].bitcast(f32),
                    start=(b == 0),
                    stop=(b == BL - 1),
                )
        # + t*bvec + 1*mask, merged as one K=2 rank-2 update:
        # st2 row0 = t (pairs bvec), row1 = ones (pairs mask)
        t_col = work.tile([BH, 1], f32)
        nc.vector.tensor_copy(t_col, ps_t)
        ps_tr = pp.tile([1, BH], f32, tag="T")
        nc.tensor.transpose(ps_tr, t_col, ident_sb[0:BH, 0:BH])
        st2 = work.tile([2, BH], f32r)
        nc.vector.memset(st2.bitcast(f32), 1.0)
        nc.vector.tensor_copy(st2[0:1, :], ps_tr)
        nc.tensor.matmul(ps_l[:, 0:512], st2, bm2_sb[:, 0:512], start=False, stop=True)
        nc.tensor.matmul(ps_l[:, 512:1024], st2, bm2_sb[:, 512:1024], start=False, stop=True)

        # ---- softmax ----
        negmax = work.tile([BH, 1], f32)
        nc.vector.tensor_reduce(
            negmax, ps_l, axis=mybir.AxisListType.X, op=ALU.max, negate=True
        )
        et = work.tile([BH, MAX_SEQ], f32)
        ssum = work.tile([BH, 1], f32)
        nc.scalar.activation(et, ps_l, AF.Exp, bias=negmax, scale=1.0, accum_out=ssum)
        rsum = work.tile([BH, 1], f32)
        nc.vector.reciprocal(rsum, ssum)
        p3 = work.tile([BH, MAX_SEQ], f32)
        sp3 = work.tile([BH, 1], f32)
        nc.vector.scalar_tensor_tensor(
            out=p3,
            in0=et,
            scalar=rsum,
            in1=aoh_bc,
            op0=ALU.mult,
            op1=ALU.mult,
            accum_out=sp3,
        )
        # c = 1 - sum(p3)  (the oh-weighted prob mass)
        c_sb = work.tile([BH, 1], f32)
        nc.vector.tensor_scalar(
            out=c_sb, in0=sp3, scalar1=-1.0, scalar2=1.0, op0=ALU.mult, op1=ALU.add
        )

        # ---- transpose p3 -> pT [128 s, 8 c, 64 bh] ----
        pT = work.tile([128, NCHUNK, BH], f32r)
        for cch in range(NCHUNK):
            ps_pt = pp.tile([128, BH], f32, tag="P", bufs=2)
            nc.tensor.transpose(
                ps_pt, p3[:, 128 * cch : 128 * (cch + 1)], ident_sb[0:BH, 0:BH]
            )
            nc.vector.tensor_copy(pT[:, cch, :], ps_pt)

        # ---- AV: psum_o [128 d, 64 bh]; stationary = V chunk, moving = pT cols ----
        ps_o = pp.tile([128, BH], f32, tag="V")
        for j in range(BL // 4):
            vc_sb = vpool.tile([128, 4, NCHUNK, HEAD_DIM], f32r, tag="vc")
            nc.sync.dma_start(
                out=vc_sb, in_=vc[4 * j : 4 * j + 4].rearrange("b (c p) d -> p b c d", p=128).bitcast(f32r)
            )
            for i in range(4):
                b = 4 * j + i
                for cch in range(NCHUNK):
                    nc.tensor.matmul(
                        ps_o[:, 4 * b : 4 * b + 4],
                        vc_sb[:, i, cch, :],
                        pT[:, cch, 4 * b : 4 * b + 4],
                        start=(b == 0 and cch == 0),
                        stop=False,
                    )
        # + c * v_new via selector matmul accumulated into ps_o
        ps_cr = pp.tile([1, BH], f32, tag="T")
        nc.tensor.transpose(ps_cr, c_sb, ident_sb[0:BH, 0:BH])
        c_row = work.tile([1, BH], f32)
        nc.vector.tensor_copy(c_row, ps_cr)
        ps_cb = pp.tile([BL, BH], f32, tag="U")
        nc.tensor.matmul(ps_cb, ones_sb[:, 0:BL], c_row, start=True, stop=True)
        rhs_ec = work.tile([BL, BH], f32)
        nc.vector.tensor_mul(rhs_ec, esel_sb, ps_cb)
        nc.tensor.matmul(ps_o, v_new, rhs_ec, start=False, stop=True)

        attnT = work.tile([128, BH], f32r)
        nc.vector.tensor_copy(attnT, ps_o)

        wo_sb = const.tile([128, GROUPS, HIDDEN], f32r)
        nc.sync.dma_start(out=wo_sb, in_=woT.rearrange("(t p) n -> p t n", p=128).bitcast(f32r))
        out_sb = work.tile([BL, HIDDEN], f32)
        attnT_g = attnT.rearrange("p (b g) -> p g b", g=GROUPS)
        for ncb in range(4):
            ps_out = pp.tile([BL, 512], f32, tag="P", bufs=2)
            for g in range(GROUPS):
                nc.tensor.matmul(
                    ps_out,
                    attnT_g[:, g, :],
                    wo_sb[:, g, 512 * ncb : 512 * (ncb + 1)],
                    start=(g == 0),
                    stop=(g == GROUPS - 1),
                )
            nc.scalar.copy(out_sb[:, 512 * ncb : 512 * (ncb + 1)], ps_out)
        nc.sync.dma_start(out=outp, in_=out_sb)

    return nc


def _legalize_waits(nc, max_waits=1):
    """walrus in this toolchain accepts at most ONE sync wait per hardware
    instruction; hoist extras onto standalone sequencer sem-waits."""
    from concourse import mybir

    n_fix = 0
    for f in nc.m.functions:
        for blk in f.blocks:
            insts = blk.instructions
            i = 0
            while i < len(insts):
                inst = insts[i]
                si = inst.sync_info
                waits = list(si.on_wait) if si is not None else []
                if len(waits) > max_waits:
                    keep = waits[-max_waits:]
                    extra = waits[:-max_waits]
                    for k, w in enumerate(extra):
                        ev = mybir.InstEventSemaphore(
                            name=f"{inst.name}-hw{k}",
                            engine=inst.engine,
                            sync_info=mybir.SyncInfo(on_wait=[w], on_update=[]),
                            ins=[],
                            outs=[],
                        )
                        insts.insert(i, ev)
                        i += 1
                    inst.sync_info = mybir.SyncInfo(
                        on_wait=keep, on_update=list(si.on_update)
                    )
                    n_fix += 1
                i += 1
    return n_fix


def _get_nc():
    global _NC
    if _NC is None:
        _NC = _build_nc()
        _legalize_waits(_NC)
    return _NC


def _host_prep(x, position, mask, k_cache, v_cache, onehot, wq, wk, wv, wo, q_norm_w, k_norm_w):
    """Build the 8 per-core input maps (all numpy, f32)."""
    x = np.asarray(x, np.float32).reshape(B, HIDDEN)
    pos = np.float32(np.asarray(position).reshape(-1)[0])
    m = np.asarray(mask, np.float32).reshape(MAX_SEQ)
    oh = np.asarray(onehot, np.float32).reshape(MAX_SEQ)
    k_cache = np.asarray(k_cache, np.float32)
    v_cache = np.asarray(v_cache, np.float32)
    wq = np.asarray(wq, np.float32)
    wk = np.asarray(wk, np.float32)
    wv = np.asarray(wv, np.float32)
    wo = np.asarray(wo, np.float32)
    qw = np.asarray(q_norm_w, np.float32)
    kw = np.asarray(k_norm_w, np.float32)

    half = HEAD_DIM // 2
    inv_freq = (1.0 / (THETA ** (np.arange(half, dtype=np.float32) / np.float32(half)))).astype(
        np.float32
    )
    freqs = (pos * inv_freq).astype(np.float32)
    cos_v = np.cos(freqs).astype(np.float32)
    sin_v = np.sin(freqs).astype(np.float32)
    # folded rope vectors: out1 = x1*(w1*cos) - x2*(w2*sin); out2 = x2*(w2*cos) + x1*(w1*sin)
    rope_rows = []
    for w in (qw, kw):
        w1, w2 = w[:half], w[half:]
        rope_rows += [w1 * cos_v, w1 * sin_v, w2 * cos_v, w2 * sin_v]
    rope_arr = np.ascontiguousarray(np.stack(rope_rows), np.float32)

    aoh = (1.0 - oh).astype(np.float32)
    cvec = np.ascontiguousarray(
        np.stack([oh, aoh, (SCALE * oh).astype(np.float32), m]), np.float32
    )

    esel = np.zeros((BL, BH), np.float32)
    for b in range(BL):
        esel[b, GROUPS * b : GROUPS * b + GROUPS] = 1.0
    ident = np.eye(128, dtype=np.float32)

    # scale K cache columns by SCALE*(1-oh_s) (folds the blend+scale into QK)
    a_s = (SCALE * aoh).astype(np.float32)

    in_maps = []
    wqT_s, wkT_s, wvT_s, woT_s = [], [], [], []
    for h in range(TP):
        wqT_s.append(np.ascontiguousarray(wq[512 * h : 512 * h + 512, :].T))
        wkT_s.append(np.ascontiguousarray(wk[128 * h : 128 * h + 128, :].T))
        wvT_s.append(np.ascontiguousarray(wv[128 * h : 128 * h + 128, :].T))
        woT_s.append(np.ascontiguousarray(wo[:, 512 * h : 512 * h + 512].T))
    for core in range(N_CORES):
        h = core % TP
        g = core // TP
        bs = slice(BL * g, BL * g + BL)
        kcT = np.ascontiguousarray(
            k_cache[bs, h].transpose(0, 2, 1) * a_s[None, None, :]
        ).astype(np.float32)
        vcs = np.ascontiguousarray(v_cache[bs, h])
        in_maps.append(
            {
                "xT": np.ascontiguousarray(x[bs].T),
                "wqT": wqT_s[h],
                "wkT": wkT_s[h],
                "wvT": wvT_s[h],
                "woT": woT_s[h],
                "kcT": kcT,
                "vc": vcs,
                "cvec": cvec,
                "rope": rope_arr,
                "esel": esel,
                "ident": ident,
            }
        )
    return in_maps


def _combine(results):
    """Sum TP partials within each batch group, concat groups."""
    out = np.zeros((B, HIDDEN), np.float32)
    for core in range(N_CORES):
        g = core // TP
        out[BL * g : BL * g + BL] += results[core]["out"]
    return out.reshape(B, 1, HIDDEN)


def run_on_cores(in_maps, trace=False, **kw):
    from concourse.bass_utils import run_bass_kernel_spmd

    nc = _get_nc()
    return run_bass_kernel_spmd(nc, in_maps, core_ids=list(range(N_CORES)), trace=trace, **kw)


def kernel(**inputs):
    in_maps = _host_prep(**inputs)
    res = run_on_cores(in_maps)
    return _combine(res.results)


# revision 24
# speedup vs baseline: 1.5074x; 1.1137x over previous
"""Trainium2 Bass kernel for single-token GQA decoder attention.

Problem (hardcoded): B=32, T=1, HIDDEN=2048, 16 q-heads / 4 kv-heads,
head_dim=128, cache len 1024.

Sharding: 8 cores = TP-4 over kv heads x DP-2 over batch. Core c handles
kv head (c % 4) and batches [16*(c//4), 16*(c//4)+16). Each core computes a
partial output [16, 2048] through its wo column shard; the host sums the 4
TP partials per batch group and concatenates the 2 batch groups.

The one-hot cache update is folded algebraically (never materialized):
  logits_s = SCALE*(1-oh_s)*(q.K_s) + SCALE*oh_s*(q.k_new) + mask_s
  out      = sum_s p_s*(1-oh_s)*V_s + (sum_s p_s*oh_s)*v_new
"""

import math
from contextlib import ExitStack

import numpy as np

MAX_SEQ = 1024
NUM_HEADS = 16
NUM_KV_HEADS = 4
HEAD_DIM = 128
HIDDEN = 2048
GROUPS = NUM_HEADS // NUM_KV_HEADS  # 4
EPS = 1e-6
THETA = 1000000.0
SCALE = 1.0 / math.sqrt(HEAD_DIM)
B = 32
N_CORES = 8
TP = NUM_KV_HEADS  # 4
DP = N_CORES // TP  # 2
BL = B // DP  # 16 batches per core
BH = BL * GROUPS  # 64 (batch*head rows per core)
NCHUNK = MAX_SEQ // 128  # 8 s-chunks
KT = HIDDEN // 128  # 16 k-tiles for projections

_NC = None  # cached Bass program


def _build_nc():
    import concourse.bass as bass
    import concourse.tile as tile
    from concourse import mybir

    f32 = mybir.dt.float32
    AF = mybir.ActivationFunctionType
    ALU = mybir.AluOpType

    nc = bass.Bass()

    xT = nc.declare_dram_parameter("xT", [HIDDEN, BL], f32, isOutput=False)[:]
    wqT = nc.declare_dram_parameter("wqT", [HIDDEN, GROUPS * HEAD_DIM], f32, isOutput=False)[:]
    wkT = nc.declare_dram_parameter("wkT", [HIDDEN, HEAD_DIM], f32, isOutput=False)[:]
    wvT = nc.declare_dram_parameter("wvT", [HIDDEN, HEAD_DIM], f32, isOutput=False)[:]
    woT = nc.declare_dram_parameter("woT", [GROUPS * HEAD_DIM, HIDDEN], f32, isOutput=False)[:]
    kcT = nc.declare_dram_parameter("kcT", [BL, HEAD_DIM, MAX_SEQ], f32, isOutput=False)[:]
    vc = nc.declare_dram_parameter("vc", [BL, 128, NCHUNK * HEAD_DIM], f32, isOutput=False)[:]
    # cvec rows: 0=oh, 1=1-oh (aoh), 2=SCALE*oh (bvec), 3=mask
    cvec = nc.declare_dram_parameter("cvec", [4, MAX_SEQ], f32, isOutput=False)[:]
    # rope rows: q: wc1, ws1, wc2, ws2 (norm-w folded); k: same
    rope = nc.declare_dram_parameter("rope", [8, HEAD_DIM // 2], f32, isOutput=False)[:]
    esel = nc.declare_dram_parameter("esel", [BL, BH], f32, isOutput=False)[:]
    ident = nc.declare_dram_parameter("ident", [128, 128], f32, isOutput=False)[:]
    outp = nc.declare_dram_parameter("out", [BL, HIDDEN], f32, isOutput=True)[:]

    HALF = HEAD_DIM // 2

    with ExitStack() as ctx:
        tc = ctx.enter_context(tile.TileContext(nc))
        const = ctx.enter_context(tc.tile_pool(name="const", bufs=1))
        work = ctx.enter_context(tc.tile_pool(name="work", bufs=1))
        kpool = ctx.enter_context(tc.tile_pool(name="kpool", bufs=2))
        vpool = ctx.enter_context(tc.tile_pool(name="vpool", bufs=2))
        pp = ctx.enter_context(tc.tile_pool(name="pp", bufs=1, space="PSUM"))

        # ---- constant / weight loads ----
        f32r = mybir.dt.float32r
        x_sb = const.tile([128, KT, BL], f32r)
        nc.sync.dma_start(out=x_sb, in_=xT.rearrange("(t p) b -> p t b", p=128).bitcast(f32r))
        wq_sb = const.tile([128, KT, GROUPS * HEAD_DIM], f32r)
        nc.sync.dma_start(out=wq_sb, in_=wqT.rearrange("(t p) n -> p t n", p=128).bitcast(f32r))
        wk_sb = const.tile([128, KT, HEAD_DIM], f32r)
        nc.sync.dma_start(out=wk_sb, in_=wkT.rearrange("(t p) n -> p t n", p=128).bitcast(f32r))
        wv_sb = const.tile([128, KT, HEAD_DIM], f32r)
        nc.sync.dma_start(out=wv_sb, in_=wvT.rearrange("(t p) n -> p t n", p=128).bitcast(f32r))
        ident_sb = const.tile([128, 128], f32)
        nc.sync.dma_start(out=ident_sb, in_=ident)
        esel_sb = const.tile([BL, BH], f32)
        nc.sync.dma_start(out=esel_sb, in_=esel)
        # broadcast [64, 1024] copy of aoh = 1-onehot
        aoh_bc = const.tile([BH, MAX_SEQ], f32)
        nc.sync.dma_start(
            out=aoh_bc,
            in_=bass.AP(
                tensor=cvec.tensor, offset=cvec.offset + MAX_SEQ, ap=[[0, BH], [1, MAX_SEQ]]
            ),
        )
        # bvec (partition 0) + mask (partition 1) as a [2, S] tile
        bm2_sb = const.tile([2, MAX_SEQ], f32r)
        nc.sync.dma_start(out=bm2_sb, in_=cvec[2:4, :].bitcast(f32r))
        # rope vectors broadcast over BL partitions
        rope_bc = const.tile([BL, 8, HALF], f32)
        nc.sync.dma_start(
            out=rope_bc,
            in_=bass.AP(tensor=rope.tensor, offset=rope.offset, ap=[[0, BL], [HALF, 8], [1, HALF]]),
        )
        eps_sb = const.tile([BL, 1], f32)
        nc.vector.memset(eps_sb, float(EPS))
        ones_sb = const.tile([1, BH], f32)
        nc.vector.memset(ones_sb, 1.0)

        # ---- projections: Q [16,512], K/V [16,128] ----
        ps_q = pp.tile([BL, GROUPS * HEAD_DIM], f32, tag="L")
        ps_k = pp.tile([BL, HEAD_DIM], f32, tag="T")
        ps_v = pp.tile([BL, HEAD_DIM], f32, tag="U")
        for t in range(KT):
            st = t == 0
            nc.tensor.matmul(ps_q, x_sb[:, t, :], wq_sb[:, t, :], start=st, stop=t == KT - 1)
            nc.tensor.matmul(ps_k, x_sb[:, t, :], wk_sb[:, t, :], start=st, stop=t == KT - 1)
            nc.tensor.matmul(ps_v, x_sb[:, t, :], wv_sb[:, t, :], start=st, stop=t == KT - 1)

        qc = work.tile([BL, GROUPS, HEAD_DIM], f32)
        nc.vector.tensor_copy(qc.rearrange("b g d -> b (g d)"), ps_q)
        kc_new = work.tile([BL, HEAD_DIM], f32)
        nc.vector.tensor_copy(kc_new, ps_k)
        v_new = work.tile([BL, HEAD_DIM], f32)
        nc.vector.tensor_copy(v_new, ps_v)

        # ---- RMSNorm (q_norm_w/k_norm_w folded into rope vecs) ----
        ssq_q = work.tile([BL, GROUPS], f32)
        q2 = work.tile([BL, GROUPS, HEAD_DIM], f32)
        nc.vector.tensor_mul(q2, qc, qc)
        nc.vector.reduce_sum(ssq_q, q2, axis=mybir.AxisListType.X)
        rms_q = work.tile([BL, GROUPS], f32)
        nc.scalar.activation(rms_q, ssq_q, AF.Sqrt, bias=eps_sb, scale=1.0 / HEAD_DIM)
        rinv_q = work.tile([BL, GROUPS], f32)
        nc.vector.reciprocal(rinv_q, rms_q)

        k2 = work.tile([BL, HEAD_DIM], f32)
        nc.vector.tensor_mul(k2, kc_new, kc_new)
        ssq_k = work.tile([BL, 1], f32)
        nc.vector.reduce_sum(ssq_k, k2, axis=mybir.AxisListType.X)
        rms_k = work.tile([BL, 1], f32)
        nc.scalar.activation(rms_k, ssq_k, AF.Sqrt, bias=eps_sb, scale=1.0 / HEAD_DIM)
        rinv_k = work.tile([BL, 1], f32)
        nc.vector.reciprocal(rinv_k, rms_k)

        qn = work.tile([BL, GROUPS, HEAD_DIM], f32)
        for g in range(GROUPS):
            nc.vector.tensor_scalar_mul(qn[:, g, :], qc[:, g, :], rinv_q[:, g : g + 1])
        kn = work.tile([BL, HEAD_DIM], f32)
        nc.vector.tensor_scalar_mul(kn, kc_new, rinv_k)

        # ---- RoPE (head-batched, rope vecs broadcast over head dim) ----
        def rvec(row, nheads):
            return bass.AP(
                tensor=rope_bc.tensor,
                offset=rope_bc.offset + row * HALF,
                ap=[list(rope_bc.ap[0]), [0, nheads], [1, HALF]],
            )

        qr = work.tile([BL, GROUPS, HEAD_DIM], f32)
        x1 = qn[:, :, 0:HALF]
        x2 = qn[:, :, HALF:HEAD_DIM]
        t1 = work.tile([BL, GROUPS, HALF], f32, tag="rtmp1")
        t2 = work.tile([BL, GROUPS, HALF], f32, tag="rtmp2")
        nc.vector.tensor_mul(t1, x1, rvec(0, GROUPS))
        nc.vector.tensor_mul(t2, x2, rvec(3, GROUPS))
        nc.vector.tensor_sub(qr[:, :, 0:HALF], t1, t2)
        t3 = work.tile([BL, GROUPS, HALF], f32, tag="rtmp1")
        t4 = work.tile([BL, GROUPS, HALF], f32, tag="rtmp2")
        nc.vector.tensor_mul(t3, x2, rvec(2, GROUPS))
        nc.vector.tensor_mul(t4, x1, rvec(1, GROUPS))
        nc.vector.tensor_add(qr[:, :, HALF:HEAD_DIM], t3, t4)

        def rvec2(row):
            return bass.AP(
                tensor=rope_bc.tensor,
                offset=rope_bc.offset + row * HALF,
                ap=[list(rope_bc.ap[0]), [1, HALF]],
            )

        kr = work.tile([BL, HEAD_DIM], f32)
        kx1 = kn[:, 0:HALF]
        kx2 = kn[:, HALF:HEAD_DIM]
        kt1 = work.tile([BL, HALF], f32, tag="ktmp1")
        kt2 = work.tile([BL, HALF], f32, tag="ktmp2")
        nc.vector.tensor_mul(kt1, kx1, rvec2(4))
        nc.vector.tensor_mul(kt2, kx2, rvec2(7))
        nc.vector.tensor_sub(kr[:, 0:HALF], kt1, kt2)
        kt3 = work.tile([BL, HALF], f32, tag="ktmp1")
        kt4 = work.tile([BL, HALF], f32, tag="ktmp2")
        nc.vector.tensor_mul(kt3, kx2, rvec2(6))
        nc.vector.tensor_mul(kt4, kx1, rvec2(5))
        nc.vector.tensor_add(kr[:, HALF:HEAD_DIM], kt3, kt4)

        # ---- build masked qT: mq[128 d, 16 b, 64 bh] block-diag layout ----
        # transpose qr per head -> psum qT [128 d, 4 g, 16 b] (one bank)
        ps_qT = pp.tile([128, GROUPS, BL], f32, tag="U")
        for g in range(GROUPS):
            nc.tensor.transpose(ps_qT[:, g, :], qr[:, g, :], ident_sb[0:BL, 0:BL])
        mq = work.tile([128, BL, BH], f32r)
        nc.vector.memset(mq.rearrange("p b c -> p (b c)").bitcast(f32), 0.0)
        # col of (b, g) within slice b = 4b + g; src col in ps_qT = 16g + b
        for b in range(BL):
            src = bass.AP(
                tensor=ps_qT.tensor,
                offset=ps_qT.offset + b,
                ap=[list(ps_qT.ap[0]), [BL, GROUPS]],
            )
            nc.vector.tensor_copy(mq[:, b, 4 * b : 4 * b + 4], src)

        # kT: [128 d, 16 b]
        ps_kT = pp.tile([128, BL], f32, tag="T")
        nc.tensor.transpose(ps_kT, kr, ident_sb[0:BL, 0:BL])
        kT_sb = work.tile([128, BL], f32r)
        nc.vector.tensor_copy(kT_sb, ps_kT)

        # ---- QK logits + t into PSUM [64, 1024] ----
        ps_l = pp.tile([BH, MAX_SEQ], f32, tag="L")  # 2 banks
        ps_t = pp.tile([BH, 1], f32, tag="T")
        for j in range(BL // 4):  # 4-batch (2 MB) cache tiles
            kc_sb = kpool.tile([128, 4, MAX_SEQ], f32r, tag="kc")
            nc.sync.dma_start(
                out=kc_sb, in_=kcT[4 * j : 4 * j + 4].rearrange("b p s -> p b s").bitcast(f32r)
            )
            for i in range(4):
                b = 4 * j + i
                lhs = mq[:, b, :]
                nc.tensor.matmul(
                    ps_l[:, 0:512], lhs, kc_sb[:, i, 0:512], start=(b == 0), stop=False
                )
                nc.tensor.matmul(
                    ps_l[:, 512:1024], lhs, kc_sb[:, i, 512:1024], start=(b == 0), stop=False
                )
                nc.tensor.matmul(
                    ps_t,
                    lhs.bitcast(f32),
                    kT_sb[:, b : b + 1].bitcast(f32),
                    start=(b == 0),
                    stop=(b == BL - 1),
                )
        # + t*bvec + 1*mask, merged as one K=2 rank-2 update:
        # st2 row0 = t (pairs bvec), row1 = ones (pairs mask)
        t_col = work.tile([BH, 1], f32)
        nc.vector.tensor_copy(t_col, ps_t)
        ps_tr = pp.tile([1, BH], f32, tag="T")
        nc.tensor.transpose(ps_tr, t_col, ident_sb[0:BH, 0:BH])
        st2 = work.tile([2, BH], f32r)
        nc.vector.memset(st2.bitcast(f32), 1.0)
        nc.vector.tensor_copy(st2[0:1, :], ps_tr)
        nc.tensor.matmul(ps_l[:, 0:512], st2, bm2_sb[:, 0:512], start=False, stop=True)
        nc.tensor.matmul(ps_l[:, 512:1024], st2, bm2_sb[:, 512:1024], start=False, stop=True)

        # ---- softmax ----
        negmax = work.tile([BH, 1], f32)
        nc.vector.tensor_reduce(
            negmax, ps_l, axis=mybir.AxisListType.X, op=ALU.max, negate=True
        )
        et = work.tile([BH, MAX_SEQ], f32)
        ssum = work.tile([BH, 1], f32)
        nc.scalar.activation(et, ps_l, AF.Exp, bias=negmax, scale=1.0, accum_out=ssum)
        rsum = work.tile([BH, 1], f32)
        nc.vector.reciprocal(rsum, ssum)
        p3 = work.tile([BH, MAX_SEQ], f32)
        sp3 = work.tile([BH, 1], f32)
        nc.vector.scalar_tensor_tensor(
            out=p3,
            in0=et,
            scalar=rsum,
            in1=aoh_bc,
            op0=ALU.mult,
            op1=ALU.mult,
            accum_out=sp3,
        )
        # c = 1 - sum(p3)  (the oh-weighted prob mass)
        c_sb = work.tile([BH, 1], f32)
        nc.vector.tensor_scalar(
            out=c_sb, in0=sp3, scalar1=-1.0, scalar2=1.0, op0=ALU.mult, op1=ALU.add
        )

        # ---- transpose p3 -> pT [128 s, 8 c, 64 bh] ----
        pT = work.tile([128, NCHUNK, BH], f32r)
        for cch in range(NCHUNK):
            ps_pt = pp.tile([128, BH], f32, tag="P", bufs=2)
            nc.tensor.transpose(
                ps_pt, p3[:, 128 * cch : 128 * (cch + 1)], ident_sb[0:BH, 0:BH]
            )
            nc.vector.tensor_copy(pT[:, cch, :], ps_pt)

        # ---- AV: psum_o [128 d, 64 bh]; stationary = V chunk, moving = pT cols ----
        ps_o = pp.tile([128, BH], f32, tag="V")
        for j in range(BL // 4):
            vc_sb = vpool.tile([128, 4, NCHUNK, HEAD_DIM], f32r, tag="vc")
            nc.sync.dma_start(
                out=vc_sb, in_=vc[4 * j : 4 * j + 4].rearrange("b p x -> p b x").bitcast(f32r)
            )
            for i in range(4):
                b = 4 * j + i
                for cch in range(NCHUNK):
                    nc.tensor.matmul(
                        ps_o[:, 4 * b : 4 * b + 4],
                        vc_sb[:, i, cch, :],
                        pT[:, cch, 4 * b : 4 * b + 4],
                        start=(b == 0 and cch == 0),
                        stop=False,
                    )
        # + c * v_new via selector matmul accumulated into ps_o
        ps_cr = pp.tile([1, BH], f32, tag="T")
        nc.tensor.transpose(ps_cr, c_sb, ident_sb[0:BH, 0:BH])
        c_row = work.tile([1, BH], f32)
        nc.vector.tensor_copy(c_row, ps_cr)
        ps_cb = pp.tile([BL, BH], f32, tag="U")
        nc.tensor.matmul(ps_cb, ones_sb[:, 0:BL], c_row, start=True, stop=True)
        rhs_ec = work.tile([BL, BH], f32)
        nc.vector.tensor_mul(rhs_ec, esel_sb, ps_cb)
        nc.tensor.matmul(ps_o, v_new, rhs_ec, start=False, stop=True)

        attnT = work.tile([128, BH], f32r)
        nc.vector.tensor_copy(attnT, ps_o)

        wo_sb = const.tile([128, GROUPS, HIDDEN], f32r)
        nc.sync.dma_start(out=wo_sb, in_=woT.rearrange("(t p) n -> p t n", p=128).bitcast(f32r))
        out_sb = work.tile([BL, HIDDEN], f32)
        attnT_g = attnT.rearrange("p (b g) -> p g b", g=GROUPS)
        for ncb in range(4):
            ps_out = pp.tile([BL, 512], f32, tag="P", bufs=2)
            for g in range(GROUPS):
                nc.tensor.matmul(
                    ps_out,
                    attnT_g[:, g, :],
                    wo_sb[:, g, 512 * ncb : 512 * (ncb + 1)],
                    start=(g == 0),
                    stop=(g == GROUPS - 1),
                )
            nc.scalar.copy(out_sb[:, 512 * ncb : 512 * (ncb + 1)], ps_out)
        nc.sync.dma_start(out=outp, in_=out_sb)

    return nc


def _legalize_waits(nc, max_waits=1):
    """walrus in this toolchain accepts at most ONE sync wait per hardware
    instruction; hoist extras onto standalone sequencer sem-waits."""
    from concourse import mybir

    n_fix = 0
    for f in nc.m.functions:
        for blk in f.blocks:
            insts = blk.instructions
            i = 0
            while i < len(insts):
                inst = insts[i]
                si = inst.sync_info
                waits = list(si.on_wait) if si is not None else []
                if len(waits) > max_waits:
                    keep = waits[-max_waits:]
                    extra = waits[:-max_waits]
                    for k, w in enumerate(extra):
                        ev = mybir.InstEventSemaphore(
                            name=f"{inst.name}-hw{k}",
                            engine=inst.engine,
                            sync_info=mybir.SyncInfo(on_wait=[w], on_update=[]),
                            ins=[],
                            outs=[],
                        )
                        insts.insert(i, ev)
                        i += 1
                    inst.sync_info = mybir.SyncInfo(
                        on_wait=keep, on_update=list(si.on_update)
                    )
                    n_fix += 1
                i += 1
    return n_fix


def _get_nc():
    global _NC
    if _NC is None:
        _NC = _build_nc()
        _legalize_waits(_NC)
    return _NC


def _host_prep(x, position, mask, k_cache, v_cache, onehot, wq, wk, wv, wo, q_norm_w, k_norm_w):
    """Build the 8 per-core input maps (all numpy, f32)."""
    x = np.asarray(x, np.float32).reshape(B, HIDDEN)
    pos = np.float32(np.asarray(position).reshape(-1)[0])
    m = np.asarray(mask, np.float32).reshape(MAX_SEQ)
    oh = np.asarray(onehot, np.float32).reshape(MAX_SEQ)
    k_cache = np.asarray(k_cache, np.float32)
    v_cache = np.asarray(v_cache, np.float32)
    wq = np.asarray(wq, np.float32)
    wk = np.asarray(wk, np.float32)
    wv = np.asarray(wv, np.float32)
    wo = np.asarray(wo, np.float32)
    qw = np.asarray(q_norm_w, np.float32)
    kw = np.asarray(k_norm_w, np.float32)

    half = HEAD_DIM // 2
    inv_freq = (1.0 / (THETA ** (np.arange(half, dtype=np.float32) / np.float32(half)))).astype(
        np.float32
    )
    freqs = (pos * inv_freq).astype(np.float32)
    cos_v = np.cos(freqs).astype(np.float32)
    sin_v = np.sin(freqs).astype(np.float32)
    # folded rope vectors: out1 = x1*(w1*cos) - x2*(w2*sin); out2 = x2*(w2*cos) + x1*(w1*sin)
    rope_rows = []
    for w in (qw, kw):
        w1, w2 = w[:half], w[half:]
        rope_rows += [w1 * cos_v, w1 * sin_v, w2 * cos_v, w2 * sin_v]
    rope_arr = np.ascontiguousarray(np.stack(rope_rows), np.float32)

    aoh = (1.0 - oh).astype(np.float32)
    cvec = np.ascontiguousarray(
        np.stack([oh, aoh, (SCALE * oh).astype(np.float32), m]), np.float32
    )

    esel = np.zeros((BL, BH), np.float32)
    for b in range(BL):
        esel[b, GROUPS * b : GROUPS * b + GROUPS] = 1.0
    ident = np.eye(128, dtype=np.float32)

    # scale K cache columns by SCALE*(1-oh_s) (folds the blend+scale into QK)
    a_s = (SCALE * aoh).astype(np.float32)

    in_maps = []
    wqT_s, wkT_s, wvT_s, woT_s = [], [], [], []
    for h in range(TP):
        wqT_s.append(np.ascontiguousarray(wq[512 * h : 512 * h + 512, :].T))
        wkT_s.append(np.ascontiguousarray(wk[128 * h : 128 * h + 128, :].T))
        wvT_s.append(np.ascontiguousarray(wv[128 * h : 128 * h + 128, :].T))
        woT_s.append(np.ascontiguousarray(wo[:, 512 * h : 512 * h + 512].T))
    for core in range(N_CORES):
        h = core % TP
        g = core // TP
        bs = slice(BL * g, BL * g + BL)
        kcT = np.ascontiguousarray(
            k_cache[bs, h].transpose(0, 2, 1) * a_s[None, None, :]
        ).astype(np.float32)
        vcs = np.ascontiguousarray(
            v_cache[bs, h].reshape(BL, NCHUNK, 128, HEAD_DIM).transpose(0, 2, 1, 3).reshape(BL, 128, NCHUNK * HEAD_DIM)
        )
        in_maps.append(
            {
                "xT": np.ascontiguousarray(x[bs].T),
                "wqT": wqT_s[h],
                "wkT": wkT_s[h],
                "wvT": wvT_s[h],
                "woT": woT_s[h],
                "kcT": kcT,
                "vc": vcs,
                "cvec": cvec,
                "rope": rope_arr,
                "esel": esel,
                "ident": ident,
            }
        )
    return in_maps


def _combine(results):
    """Sum TP partials within each batch group, concat groups."""
    out = np.zeros((B, HIDDEN), np.float32)
    for core in range(N_CORES):
        g = core // TP
        out[BL * g : BL * g + BL] += results[core]["out"]
    return out.reshape(B, 1, HIDDEN)


def run_on_cores(in_maps, trace=False, **kw):
    from concourse.bass_utils import run_bass_kernel_spmd

    nc = _get_nc()
    return run_bass_kernel_spmd(nc, in_maps, core_ids=list(range(N_CORES)), trace=trace, **kw)


def kernel(**inputs):
    in_maps = _host_prep(**inputs)
    res = run_on_cores(in_maps)
    return _combine(res.results)
